# revision 23
# baseline (speedup 1.0000x reference)
"""Trainium2 Bass kernel for CausalTensionGraphLayer.

Math (host-fused factorization):
  a   = x @ w1[:D] + b1                        [T, H]   (H = D/2)
  c   = x @ w1[D:]                             [T, H]
  hid_w  = silu(a[t] + c[t-w-1])               (c term is 0 when t-w-1 < 0)
  tau_w  = sigmoid(hid_w @ w2 + b2)
  u   = x @ (wv_w @ m2) + wv_b @ m2            [T, D]   (m2 = merge_w[D:])
  msg2[t] = sum_w tau_w[t] * u[t-w-1]          (== (msg @ m2)[t] by linearity;
                                                u -> wv_b @ m2 when t-w-1 < 0)
  y      = x @ merge_w[:D] + msg2 + merge_b
  out    = LayerNorm(y) * gamma + beta

Folding m2 into wv on the host removes a full [T,D]x[D,D] matmul from the
device: the value projection and the merge of the message happen in one
x @ Wc pass, and the feature-major msg2 is added into the token-major y
PSUM with cheap identity-matmul transposes (4 x N=128 per 512-wide tile).

Neighbor gathers are row shifts of x, so with zero rows prepended for the
out-of-range halo the same compute path reproduces the reference exactly.

Sharding: data-parallel over the B*T = 8192 token rows, 1024 own tokens per
core plus a 4-row halo (zeros at batch boundaries). No collectives.

All device inputs are pre-packed on the host into [128, bytes] partition-
major arrays so every DMA lands as 128 contiguous multi-KB descriptors
(the previous per-(k,m)-strided layout shredded loads into ~400B pieces and
left the PE waiting on weights for ~30us). Loads are split across the two
HWDGE trigger queues (sync, scalar) in PE consumption order.
"""

from contextlib import ExitStack

import numpy as np
import ml_dtypes

import concourse.bass as bass
import concourse.bacc as bacc
import concourse.tile as tile
from concourse import mybir
from concourse.bass_utils import run_bass_kernel_spmd

BF16 = ml_dtypes.bfloat16

B, T, D = 2, 4096, 1024
H = D // 2
W = 4
EPS = 1e-5
NCORES = 8
NTOK = (B * T) // NCORES          # 1024 own tokens per core
HALO = W                          # 4
NQ = 4                            # token quarters per core
QT = NTOK // NQ                   # 256 own tokens per quarter
QG = QT + HALO                    # 260 grid cols per quarter (4 halo + 256)
KD = D // 128                     # 8 K-chunks over D
MH = H // 128                     # 4 M-tiles over H
MD = D // 128                     # 8 M-tiles over D
NT = QT // 128                    # 2 token tiles per quarter

FP32 = mybir.dt.float32
I32 = mybir.dt.int32
BF = mybir.dt.bfloat16
AF = mybir.ActivationFunctionType
ALU = mybir.AluOpType
AX = mybir.AxisListType


def build_nc(use_gamma_beta: bool, use_merge_b: bool, use_b1: bool,
             use_bc: bool):
    nc = bacc.Bacc(None, target_bir_lowering=False)

    # Host-packed inputs: every tensor arrives as [128, free] with the
    # exact per-partition byte layout of its SBUF tile.
    xq_d = [nc.dram_tensor(f"xq{q}", [128, KD, QG], BF, kind="ExternalInput")
            for q in range(NQ)]
    w1a = nc.dram_tensor("w1a", [128, MH, KD, 128], BF, kind="ExternalInput")
    w1c = nc.dram_tensor("w1c", [128, MH, KD, 128], BF, kind="ExternalInput")
    wc = nc.dram_tensor("wc", [128, MD, KD, 128], BF, kind="ExternalInput")
    m1 = nc.dram_tensor("m1", [128, KD, D], BF, kind="ExternalInput")
    w2rep = nc.dram_tensor("w2rep", [128, MH, 128], BF, kind="ExternalInput")
    b2r = nc.dram_tensor("b2r", [128, 1], FP32, kind="ExternalInput")
    ident = nc.dram_tensor("ident", [128, 128], BF, kind="ExternalInput")
    if use_b1:
        b1r = nc.dram_tensor("b1r", [128, MH], FP32, kind="ExternalInput")
    if use_bc:
        bcr = nc.dram_tensor("bcr", [128, MD], FP32, kind="ExternalInput")
    if use_gamma_beta:
        gam = nc.dram_tensor("gam", [1, D], FP32, kind="ExternalInput")
        bet = nc.dram_tensor("bet", [1, D], FP32, kind="ExternalInput")
    if use_merge_b:
        mbt = nc.dram_tensor("mbt", [1, D], FP32, kind="ExternalInput")
    y = nc.dram_tensor("y", [NTOK, D], BF, kind="ExternalOutput")

    with tile.TileContext(nc) as tc, ExitStack() as ctx:
        persist = ctx.enter_context(tc.tile_pool(name="persist", bufs=1))
        abpool = ctx.enter_context(tc.tile_pool(name="abpool", bufs=NQ))
        qpool = ctx.enter_context(tc.tile_pool(name="qpool", bufs=2))
        mpool = ctx.enter_context(tc.tile_pool(name="mpool", bufs=4))
        mpool2 = ctx.enter_context(tc.tile_pool(name="mpool2", bufs=2))
        hpool = ctx.enter_context(tc.tile_pool(name="hpool", bufs=2 * NQ))
        opool = ctx.enter_context(tc.tile_pool(name="opool", bufs=3))
        ps_acc = ctx.enter_context(tc.tile_pool(name="ps_acc", bufs=3, space="PSUM"))
        ps_y = ctx.enter_context(tc.tile_pool(name="ps_y", bufs=4, space="PSUM"))

        # ---- persistent tiles ----
        xq_sb = [
            persist.tile([128, KD, QG], BF, tag=f"xq{q}", name=f"xq{q}")
            for q in range(NQ)
        ]
        w1a_sb = persist.tile([128, MH, KD, 128], BF, tag="w1a")
        w1c_sb = persist.tile([128, MH, KD, 128], BF, tag="w1c")
        wc_sb = persist.tile([128, MD, KD, 128], BF, tag="wc")
        m1_sb = persist.tile([128, KD, D], BF, tag="m1")
        w2rep_sb = persist.tile([128, MH, 128], BF, tag="w2rep")
        b2_sb = persist.tile([128, 1], FP32, tag="b2")
        id_sb = persist.tile([128, 128], BF, tag="ident")

        # ---- loads: ALL input triggers on the sync queue (no compute ops
        # there, so a compute op stuck on a semaphore can never delay a
        # later input load), ordered by PE first-consumption time ----
        nc.sync.dma_start(out=xq_sb[0][:, 0:1, :], in_=xq_d[0][:, 0:1, :])
        nc.sync.dma_start(out=w1a_sb[:, 0, :, :], in_=w1a[:, 0, :, :])
        nc.sync.dma_start(out=xq_sb[0][:, 1:4, :], in_=xq_d[0][:, 1:4, :])
        nc.sync.dma_start(out=xq_sb[0][:, 4:KD, :], in_=xq_d[0][:, 4:KD, :])
        for mm in range(1, MH):
            nc.sync.dma_start(out=w1a_sb[:, mm, :, :], in_=w1a[:, mm, :, :])
        if use_b1:
            b1_sb = persist.tile([128, MH], FP32, tag="b1")
            nc.sync.dma_start(out=b1_sb, in_=b1r[:, :])
        nc.sync.dma_start(out=xq_sb[1][:, :, :], in_=xq_d[1][:, :, :])
        nc.sync.dma_start(out=w1c_sb[:, 0:2, :, :], in_=w1c[:, 0:2, :, :])
        nc.sync.dma_start(out=xq_sb[2][:, :, :], in_=xq_d[2][:, :, :])
        nc.sync.dma_start(out=w1c_sb[:, 2:MH, :, :], in_=w1c[:, 2:MH, :, :])
        nc.sync.dma_start(out=xq_sb[3][:, :, :], in_=xq_d[3][:, :, :])
        nc.sync.dma_start(out=wc_sb[:, 0:2, :, :], in_=wc[:, 0:2, :, :])
        nc.sync.dma_start(out=wc_sb[:, 2:4, :, :], in_=wc[:, 2:4, :, :])
        nc.sync.dma_start(out=w2rep_sb[:, :, :], in_=w2rep[:, :, :])
        nc.sync.dma_start(out=b2_sb, in_=b2r[:, :])
        nc.sync.dma_start(out=id_sb, in_=ident[:, :])
        nc.sync.dma_start(out=wc_sb[:, 4:6, :, :], in_=wc[:, 4:6, :, :])
        nc.sync.dma_start(out=wc_sb[:, 6:MD, :, :], in_=wc[:, 6:MD, :, :])
        if use_bc:
            bc_sb = persist.tile([128, MD], FP32, tag="bc")
            nc.sync.dma_start(out=bc_sb, in_=bcr[:, :])
        for kk in range(0, KD, 2):
            nc.sync.dma_start(out=m1_sb[:, kk:kk + 2, :], in_=m1[:, kk:kk + 2, :])
        magic_sb = persist.tile([128, 1], I32, tag="magic")
        nc.vector.memset(magic_sb, 0x5F3759DF)
        one_i = persist.tile([128, 1], I32, tag="onei")
        nc.vector.memset(one_i, 1)
        if use_gamma_beta:
            gam_sb = persist.tile([128, D], FP32, tag="gam")
            nc.sync.dma_start(out=gam_sb, in_=gam.partition_broadcast(128))
            bet_sb = persist.tile([128, D], FP32, tag="bet")
            nc.sync.dma_start(out=bet_sb, in_=bet.partition_broadcast(128))
        if use_merge_b:
            mb_sb = persist.tile([128, D], FP32, tag="mb")
            nc.sync.dma_start(out=mb_sb, in_=mbt.partition_broadcast(128))

        # ---- stage 1a: a (own tokens) and c (shifted grid) per quarter,
        #      with the gating elementwise prep trailing on DVE/ScalarE ----
        # CG = cols of c/u actually consumed by a quarter's gating windows.
        CG = QT + HALO - 1            # 259
        aqs, cqs, uqs, hsss = [], [], [], []
        for q in range(NQ):
            xs = xq_sb[q]
            aq = abpool.tile([128, MH, QT], BF, tag="aq")
            aqs.append(aq)
            cq = abpool.tile([128, MH, CG], BF, tag="cq")
            cqs.append(cq)
            for m in range(MH):
                ps = ps_acc.tile([128, QT], FP32, tag="acc")
                for k in range(KD):
                    nc.tensor.matmul(
                        ps, w1a_sb[:, m, k, :], xs[:, k, HALO:HALO + QT],
                        start=(k == 0), stop=(k == KD - 1),
                    )
                if use_b1:
                    nc.scalar.activation(
                        out=aq[:, m, :], in_=ps, func=AF.Identity,
                        bias=b1_sb[:, m:m + 1], scale=1.0,
                    )
                else:
                    nc.scalar.activation(out=aq[:, m, :], in_=ps, func=AF.Copy)
            for m in range(MH):
                ps = ps_acc.tile([128, CG], FP32, tag="acc")
                for k in range(KD):
                    nc.tensor.matmul(
                        ps, w1c_sb[:, m, k, :], xs[:, k, 0:CG],
                        start=(k == 0), stop=(k == KD - 1),
                    )
                nc.scalar.activation(out=cq[:, m, :], in_=ps, func=AF.Copy)
            # hid = silu(a + c_shift) directly on ScalarE (silu_and_others
            # set also covers tanh/copy/square -> single table load).
            hq = []
            for p in range(W // 2):
                hs = mpool2.tile([128, MH, 2, QT], BF, tag="hs")
                for wi in range(2):
                    w = 2 * p + wi
                    o = HALO - 1 - w
                    nc.vector.tensor_add(hs[:, :, wi, :], aq, cq[:, :, o:o + QT])
                hss = hpool.tile([128, MH, 2, QT], BF, tag="hss")
                nc.scalar.activation(out=hss, in_=hs, func=AF.Silu)
                hq.append(hss)
            hsss.append(hq)
        # ---- stage 1b: u = x @ Wc (+ wv_b @ m2), shifted grid ----
        for q in range(NQ):
            xs = xq_sb[q]
            uq = abpool.tile([128, MD, CG], BF, tag="uq")
            uqs.append(uq)
            for m in range(MD):
                ps = ps_acc.tile([128, CG], FP32, tag="acc")
                for k in range(KD):
                    nc.tensor.matmul(
                        ps, wc_sb[:, m, k, :], xs[:, k, 0:CG],
                        start=(k == 0), stop=(k == KD - 1),
                    )
                if use_bc:
                    nc.scalar.activation(
                        out=uq[:, m, :], in_=ps, func=AF.Identity,
                        bias=bc_sb[:, m:m + 1], scale=1.0,
                    )
                else:
                    nc.vector.tensor_scalar_mul(uq[:, m, :], ps, 1.0)
        # ---- stage 2: per quarter gate -> msg2 -> y -> LN -> store ----
        for q in range(NQ):
            aq, cq, uq = aqs[q], cqs[q], uqs[q]
            tauq = qpool.tile([128, W, QT], BF, tag="tauq")
            for p in range(W // 2):
                hss = hsss[q][p]
                pl = ps_y.tile([128, 2 * QT], FP32, tag="logit", bufs=1)
                for k in range(MH):
                    nc.tensor.matmul(
                        pl, w2rep_sb[:, k, :], hss[:, k, :, :],
                        start=(k == 0), stop=(k == MH - 1),
                    )
                # tau = sigmoid(logit + b2) = 0.5*tanh(0.5*(logit + b2)) + 0.5
                # (tanh lives in the silu table set; sigmoid does not).
                # b2r is pre-halved on the host; affine runs 4x on DVE.
                traw = mpool2.tile([128, 2, QT], BF, tag="traw")
                nc.scalar.activation(
                    out=traw, in_=pl.rearrange("p (a b) -> p a b", a=2),
                    func=AF.Tanh, bias=b2_sb[:, 0:1], scale=0.5,
                )
                nc.vector.tensor_scalar(
                    out=tauq[:, 2 * p:2 * p + 2, :], in0=traw,
                    scalar1=0.5, scalar2=0.5, op0=ALU.mult, op1=ALU.add,
                )
            # msg2 = sum_w tau_w * shift(u, w+1): tau broadcast over the
            # 8 d-tiles via a step-0 mid dimension. Computed per 128-token
            # tile so the first transpose matmuls unblock after half the
            # DVE chain.
            msgq = qpool.tile([128, MD, QT], BF, tag="msgq")

            def tau_b(w, t0, tauq=tauq):
                s = tauq[:, w, t0:t0 + 128]
                return bass.AP(
                    tensor=s.tensor, offset=s.offset,
                    ap=[s.ap[0], [0, MD], s.ap[1]],
                )

            for tt in range(NT):
                t0 = 128 * tt
                pw = []
                for w in range(W):
                    o = HALO - 1 - w
                    pt = mpool.tile([128, MD, 128], BF, tag="pw")
                    nc.vector.tensor_mul(
                        pt, tau_b(w, t0), uq[:, :, o + t0:o + t0 + 128]
                    )
                    pw.append(pt)
                    if w == 1:
                        m01 = mpool.tile([128, MD, 128], BF, tag="pw")
                        nc.vector.tensor_add(m01, pw[0], pw[1])
                nc.vector.tensor_add(pw[3], pw[2], pw[3])
                nc.vector.tensor_add(msgq[:, :, t0:t0 + 128], m01, pw[3])
            # y = x @ m1 + msg2^T (+ merge_b), token-major via PSUM:
            # identity-rhs matmuls transpose-accumulate msg2 into the m1 sum.
            # The last quarter runs per token tile (separate stat tiles) so
            # tt0's LayerNorm + store overlap tt1's matmuls -> shorter tail.
            g0 = q * QT
            tt_groups = [[0, 1]] if q < NQ - 1 else [[0], [1]]
            for group in tt_groups:
                nt = len(group)
                srow = mpool.tile([128, nt, 2], FP32, tag="srow", name="srow")
                sqs = mpool.tile([128, nt, 2], FP32, tag="sqs", name="sqs")
                ysb = {}
                for gi, tt in enumerate(group):
                    yt = opool.tile([128, D], FP32, tag="ysb", name="ysb")
                    ysb[tt] = yt
                    for half in range(2):
                        n0 = half * 512
                        yps = ps_y.tile([128, 512], FP32, tag="y", name="yps")
                        for k in range(KD):
                            nc.tensor.matmul(
                                yps,
                                xq_sb[q][:, k, HALO + 128 * tt:HALO + 128 * tt + 128],
                                m1_sb[:, k, n0:n0 + 512],
                                start=(k == 0), stop=False,
                            )
                        for j in range(4):
                            f = 4 * half + j
                            nc.tensor.matmul(
                                yps[:, 128 * j:128 * j + 128],
                                msgq[:, f, 128 * tt:128 * tt + 128],
                                id_sb,
                                start=False, stop=True,
                            )
                        if use_merge_b:
                            nc.vector.tensor_add(yps, yps, mb_sb[:, n0:n0 + 512])
                        # Evict PSUM while collecting LN stats: Copy gives
                        # sum(y), Square gives sum(y^2); both in silu set.
                        nc.scalar.activation(
                            out=yt[:, n0:n0 + 512], in_=yps, func=AF.Copy,
                            accum_out=srow[:, gi, half:half + 1],
                        )
                        junk = mpool2.tile([128, 512], FP32, tag="junk")
                        nc.scalar.activation(
                            out=junk, in_=yps, func=AF.Square,
                            accum_out=sqs[:, gi, half:half + 1],
                        )
                # LayerNorm finalize; rstd via bit-trick seed + 2 Newton
                # steps (keeps sqrt off ScalarE; GpSimd/Pool rejects the
                # per-partition-scalar op forms, so the smalls stay on DVE
                # and the big apply rides ScalarE as Identity(scale,bias)).
                ve = nc.vector
                ssum = mpool.tile([128, nt], FP32, tag="ssum", name="ssum")
                ve.tensor_add(ssum, srow[:, :, 0], srow[:, :, 1])
                qsum = mpool.tile([128, nt], FP32, tag="qsum", name="qsum")
                ve.tensor_add(qsum, sqs[:, :, 0], sqs[:, :, 1])
                mean = mpool.tile([128, nt], FP32, tag="mean", name="mean")
                ve.tensor_scalar_mul(mean, ssum, 1.0 / D)
                m2e = mpool.tile([128, nt], FP32, tag="m2e", name="m2e")
                ve.scalar_tensor_tensor(   # mean^2 - eps
                    out=m2e, in0=mean, scalar=1.0, in1=mean,
                    op0=ALU.mult, op1=ALU.mult,
                )
                ve.tensor_scalar_add(m2e, m2e, -EPS)
                veps = mpool.tile([128, nt], FP32, tag="veps", name="veps")
                ve.scalar_tensor_tensor(   # q/D - (mean^2 - eps)
                    out=veps, in0=qsum, scalar=1.0 / D, in1=m2e,
                    op0=ALU.mult, op1=ALU.subtract,
                )
                rbits = mpool.tile([128, nt], I32, tag="rbits", name="rbits")
                ve.tensor_scalar(
                    out=rbits, in0=veps.bitcast(I32), scalar1=one_i[:, 0:1],
                    scalar2=None, op0=ALU.arith_shift_right,
                )
                ve.tensor_tensor(
                    out=rbits, in0=magic_sb.to_broadcast([128, nt]), in1=rbits,
                    op=ALU.subtract,
                )
                rstd = rbits.bitcast(FP32)
                for _ in range(2):
                    nt1 = mpool.tile([128, nt], FP32, tag="nt1", name="nt1")
                    ve.tensor_mul(nt1, rstd, rstd)
                    ve.tensor_mul(nt1, nt1, veps)
                    ve.tensor_scalar(
                        out=nt1, in0=nt1, scalar1=-0.5, scalar2=1.5,
                        op0=ALU.mult, op1=ALU.add,
                    )
                    ve.tensor_mul(rstd, rstd, nt1)
                nmr = mpool.tile([128, nt], FP32, tag="nmr", name="nmr")
                ve.scalar_tensor_tensor(   # -mean * rstd
                    out=nmr, in0=mean, scalar=-1.0, in1=rstd,
                    op0=ALU.mult, op1=ALU.mult,
                )
                for gi, tt in enumerate(group):
                    tok0 = g0 + 128 * tt
                    yo = opool.tile([128, D], BF, tag="yout", name="yout")
                    nc.scalar.activation(
                        out=yo, in_=ysb[tt], func=AF.Identity,
                        bias=nmr[:, gi:gi + 1], scale=rstd[:, gi:gi + 1],
                    )
                    if use_gamma_beta:
                        nc.vector.tensor_mul(yo, yo, gam_sb)
                        nc.vector.tensor_add(yo, yo, bet_sb)
                    nc.sync.dma_start(out=y[tok0:tok0 + 128, :], in_=yo)
    nc.compile()
    return nc


_CACHE: dict = {}


def _get_nc(flags):
    if flags not in _CACHE:
        _CACHE[flags] = build_nc(*flags)
    return _CACHE[flags]


def _pack_km(wmat, mt):
    """[D, mt*128] weight -> [128, mt, KD, 128] m-outer partition-major."""
    # w[k*128+p, m*128+c] -> out[p, m, k, c]
    wr = wmat.reshape(KD, 128, mt, 128)
    return np.ascontiguousarray(wr.transpose(1, 2, 0, 3)).astype(BF16)


def kernel(x, w1, b1, w2, b2, wv_w, wv_b, merge_w, merge_b, gamma, beta):
    x = np.asarray(x, dtype=np.float32)
    w1 = np.asarray(w1, dtype=np.float32)
    b1 = np.asarray(b1, dtype=np.float32)
    w2 = np.asarray(w2, dtype=np.float32)
    b2 = np.asarray(b2, dtype=np.float32)
    wv_w = np.asarray(wv_w, dtype=np.float32)
    wv_b = np.asarray(wv_b, dtype=np.float32)
    merge_w = np.asarray(merge_w, dtype=np.float32)
    merge_b = np.asarray(merge_b, dtype=np.float32)
    gamma = np.asarray(gamma, dtype=np.float32)
    beta = np.asarray(beta, dtype=np.float32)

    m2w = merge_w[D:]                       # [D, D]
    wc = wv_w @ m2w                         # fused value+merge projection
    bc = wv_b @ m2w                         # [D]
    use_gamma_beta = not (np.all(gamma == 1.0) and np.all(beta == 0.0))
    use_merge_b = bool(np.any(merge_b != 0.0))
    use_b1 = bool(np.any(b1 != 0.0))
    use_bc = bool(np.any(bc != 0.0))
    flags = (use_gamma_beta, use_merge_b, use_b1, use_bc)
    nc = _get_nc(flags)

    x2 = x.reshape(B * T, D)
    shared = {
        "w1a": _pack_km(w1[:D], MH),
        "w1c": _pack_km(w1[D:], MH),
        "wc": _pack_km(wc, MD),
        "m1": np.ascontiguousarray(
            merge_w[:D].reshape(KD, 128, D).transpose(1, 0, 2)
        ).astype(BF16),
        "w2rep": np.ascontiguousarray(
            np.broadcast_to(w2.reshape(MH, 128, 1), (MH, 128, 128))
            .transpose(1, 0, 2)
        ).astype(BF16),
        # pre-halved: tau = 0.5*tanh(0.5*logit + 0.5*b2) + 0.5
        "b2r": np.full((128, 1), 0.5 * float(b2[0]), np.float32),
        "ident": np.eye(128, dtype=BF16),
    }
    if use_b1:
        shared["b1r"] = np.ascontiguousarray(b1.reshape(MH, 128).T)
    if use_bc:
        shared["bcr"] = np.ascontiguousarray(bc.reshape(MD, 128).T)
    if use_gamma_beta:
        shared["gam"] = gamma.reshape(1, D)
        shared["bet"] = beta.reshape(1, D)
    if use_merge_b:
        shared["mbt"] = merge_b.reshape(1, D)

    in_maps = []
    for c in range(NCORES):
        t0 = c * NTOK
        xs = np.zeros((NTOK + HALO, D), np.float32)
        xs[HALO:] = x2[t0:t0 + NTOK]
        if t0 % T != 0:  # halo stays inside the same batch element
            xs[:HALO] = x2[t0 - HALO:t0]
        xsT = xs.T.reshape(KD, 128, NTOK + HALO)  # [k, p, grid]
        m = dict(shared)
        for q in range(NQ):
            g0 = q * QT
            m[f"xq{q}"] = np.ascontiguousarray(
                xsT[:, :, g0:g0 + QG].transpose(1, 0, 2)
            ).astype(BF16)
        in_maps.append(m)

    res = run_bass_kernel_spmd(nc, in_maps, core_ids=list(range(NCORES)))
    out = np.concatenate(
        [np.asarray(r["y"]).astype(np.float32) for r in res.results], axis=0
    )
    return out.reshape(B, T, D)


# revision 25
# speedup vs baseline: 1.2101x; 1.2101x over previous
"""Trainium2 Bass kernel for CausalTensionGraphLayer.

Math (host-fused factorization):
  a   = x @ w1[:D] + b1                        [T, H]   (H = D/2)
  c   = x @ w1[D:]                             [T, H]
  hid_w  = silu(a[t] + c[t-w-1])               (c term is 0 when t-w-1 < 0)
  tau_w  = sigmoid(hid_w @ w2 + b2)
  u   = x @ (wv_w @ m2) + wv_b @ m2            [T, D]   (m2 = merge_w[D:])
  msg2[t] = sum_w tau_w[t] * u[t-w-1]          (== (msg @ m2)[t] by linearity;
                                                u -> wv_b @ m2 when t-w-1 < 0)
  y      = x @ merge_w[:D] + msg2 + merge_b
  out    = LayerNorm(y) * gamma + beta

Folding m2 into wv on the host removes a full [T,D]x[D,D] matmul from the
device: the value projection and the merge of the message happen in one
x @ Wc pass, and the feature-major msg2 is added into the token-major y
PSUM with cheap identity-matmul transposes (4 x N=128 per 512-wide tile).

Neighbor gathers are row shifts of x, so with zero rows prepended for the
out-of-range halo the same compute path reproduces the reference exactly.

Sharding: data-parallel over the B*T = 8192 token rows, 1024 own tokens per
core plus a 4-row halo (zeros at batch boundaries). No collectives.

All device inputs are pre-packed on the host into [128, bytes] partition-
major arrays so every DMA lands as 128 contiguous multi-KB descriptors
(the previous per-(k,m)-strided layout shredded loads into ~400B pieces and
left the PE waiting on weights for ~30us). Loads are split across the two
HWDGE trigger queues (sync, scalar) in PE consumption order.
"""

from contextlib import ExitStack

import numpy as np
import ml_dtypes

import concourse.bass as bass
import concourse.bacc as bacc
import concourse.tile as tile
from concourse import mybir
from concourse.bass_utils import run_bass_kernel_spmd

BF16 = ml_dtypes.bfloat16

B, T, D = 2, 4096, 1024
H = D // 2
W = 4
EPS = 1e-5
NCORES = 8
NTOK = (B * T) // NCORES          # 1024 own tokens per core
HALO = W                          # 4
NQ = 4                            # token quarters per core
QT = NTOK // NQ                   # 256 own tokens per quarter
QG = QT + HALO                    # 260 grid cols per quarter (4 halo + 256)
KD = D // 128                     # 8 K-chunks over D
MH = H // 128                     # 4 M-tiles over H
MD = D // 128                     # 8 M-tiles over D
NT = QT // 128                    # 2 token tiles per quarter

FP32 = mybir.dt.float32
I32 = mybir.dt.int32
BF = mybir.dt.bfloat16
AF = mybir.ActivationFunctionType
ALU = mybir.AluOpType
AX = mybir.AxisListType


def build_nc(use_gamma_beta: bool, use_merge_b: bool, use_b1: bool,
             use_bc: bool):
    nc = bacc.Bacc(None, target_bir_lowering=False)

    # Host-packed inputs: every tensor arrives as [128, free] with the
    # exact per-partition byte layout of its SBUF tile.
    xq_d = [nc.dram_tensor(f"xq{q}", [128, KD, QG], BF, kind="ExternalInput")
            for q in range(NQ)]
    w1a = nc.dram_tensor("w1a", [128, MH, KD, 128], BF, kind="ExternalInput")
    w1c = nc.dram_tensor("w1c", [128, MH, KD, 128], BF, kind="ExternalInput")
    wc = nc.dram_tensor("wc", [128, MD, KD, 128], BF, kind="ExternalInput")
    m1 = nc.dram_tensor("m1", [128, KD, D], BF, kind="ExternalInput")
    w2rep = nc.dram_tensor("w2rep", [128, MH, 128], BF, kind="ExternalInput")
    b2r = nc.dram_tensor("b2r", [128, 1], FP32, kind="ExternalInput")
    ident = nc.dram_tensor("ident", [128, 128], BF, kind="ExternalInput")
    if use_b1:
        b1r = nc.dram_tensor("b1r", [128, MH], FP32, kind="ExternalInput")
    if use_bc:
        bcr = nc.dram_tensor("bcr", [128, MD], FP32, kind="ExternalInput")
    if use_gamma_beta:
        gam = nc.dram_tensor("gam", [1, D], FP32, kind="ExternalInput")
        bet = nc.dram_tensor("bet", [1, D], FP32, kind="ExternalInput")
    if use_merge_b:
        mbt = nc.dram_tensor("mbt", [1, D], FP32, kind="ExternalInput")
    y = nc.dram_tensor("y", [NTOK, D], BF, kind="ExternalOutput")

    with tile.TileContext(nc) as tc, ExitStack() as ctx:
        persist = ctx.enter_context(tc.tile_pool(name="persist", bufs=1))
        abpool = ctx.enter_context(tc.tile_pool(name="abpool", bufs=NQ))
        qpool = ctx.enter_context(tc.tile_pool(name="qpool", bufs=2))
        mpool = ctx.enter_context(tc.tile_pool(name="mpool", bufs=4))
        mpool2 = ctx.enter_context(tc.tile_pool(name="mpool2", bufs=2))
        hpool = ctx.enter_context(tc.tile_pool(name="hpool", bufs=2 * NQ))
        opool = ctx.enter_context(tc.tile_pool(name="opool", bufs=4))
        ps_acc = ctx.enter_context(tc.tile_pool(name="ps_acc", bufs=3, space="PSUM"))
        ps_y = ctx.enter_context(tc.tile_pool(name="ps_y", bufs=4, space="PSUM"))

        # ---- persistent tiles ----
        xq_sb = [
            persist.tile([128, KD, QG], BF, tag=f"xq{q}", name=f"xq{q}")
            for q in range(NQ)
        ]
        w1a_sb = persist.tile([128, MH, KD, 128], BF, tag="w1a")
        w1c_sb = persist.tile([128, MH, KD, 128], BF, tag="w1c")
        wc_sb = persist.tile([128, MD, KD, 128], BF, tag="wc")
        m1_sb = persist.tile([128, KD, D], BF, tag="m1")
        w2rep_sb = persist.tile([128, MH, 128], BF, tag="w2rep")
        b2_sb = persist.tile([128, 1], FP32, tag="b2")
        id_sb = persist.tile([128, 128], BF, tag="ident")

        # ---- loads: ALL input triggers on the sync queue (no compute ops
        # there, so a compute op stuck on a semaphore can never delay a
        # later input load), ordered by PE first-consumption time ----
        nc.sync.dma_start(out=xq_sb[0][:, 0:1, :], in_=xq_d[0][:, 0:1, :])
        nc.sync.dma_start(out=w1a_sb[:, 0, :, :], in_=w1a[:, 0, :, :])
        nc.sync.dma_start(out=xq_sb[0][:, 1:4, :], in_=xq_d[0][:, 1:4, :])
        nc.sync.dma_start(out=xq_sb[0][:, 4:KD, :], in_=xq_d[0][:, 4:KD, :])
        for mm in range(1, MH):
            nc.sync.dma_start(out=w1a_sb[:, mm, :, :], in_=w1a[:, mm, :, :])
        if use_b1:
            b1_sb = persist.tile([128, MH], FP32, tag="b1")
            nc.sync.dma_start(out=b1_sb, in_=b1r[:, :])
        nc.sync.dma_start(out=xq_sb[1][:, :, :], in_=xq_d[1][:, :, :])
        nc.sync.dma_start(out=w1c_sb[:, 0:2, :, :], in_=w1c[:, 0:2, :, :])
        nc.sync.dma_start(out=xq_sb[2][:, :, :], in_=xq_d[2][:, :, :])
        nc.sync.dma_start(out=w1c_sb[:, 2:MH, :, :], in_=w1c[:, 2:MH, :, :])
        nc.sync.dma_start(out=xq_sb[3][:, :, :], in_=xq_d[3][:, :, :])
        nc.sync.dma_start(out=wc_sb[:, 0:2, :, :], in_=wc[:, 0:2, :, :])
        nc.sync.dma_start(out=wc_sb[:, 2:4, :, :], in_=wc[:, 2:4, :, :])
        nc.sync.dma_start(out=w2rep_sb[:, :, :], in_=w2rep[:, :, :])
        nc.sync.dma_start(out=b2_sb, in_=b2r[:, :])
        nc.sync.dma_start(out=id_sb, in_=ident[:, :])
        nc.sync.dma_start(out=wc_sb[:, 4:6, :, :], in_=wc[:, 4:6, :, :])
        nc.sync.dma_start(out=wc_sb[:, 6:MD, :, :], in_=wc[:, 6:MD, :, :])
        if use_bc:
            bc_sb = persist.tile([128, MD], FP32, tag="bc")
            nc.sync.dma_start(out=bc_sb, in_=bcr[:, :])
        for kk in range(0, KD, 2):
            nc.sync.dma_start(out=m1_sb[:, kk:kk + 2, :], in_=m1[:, kk:kk + 2, :])
        magic_sb = persist.tile([128, 1], I32, tag="magic")
        nc.vector.memset(magic_sb, 0x5F3759DF)
        one_i = persist.tile([128, 1], I32, tag="onei")
        nc.vector.memset(one_i, 1)
        if use_gamma_beta:
            gam_sb = persist.tile([128, D], FP32, tag="gam")
            nc.sync.dma_start(out=gam_sb, in_=gam.partition_broadcast(128))
            bet_sb = persist.tile([128, D], FP32, tag="bet")
            nc.sync.dma_start(out=bet_sb, in_=bet.partition_broadcast(128))
        if use_merge_b:
            mb_sb = persist.tile([128, D], FP32, tag="mb")
            nc.sync.dma_start(out=mb_sb, in_=mbt.partition_broadcast(128))

        # ---- stage 1a: a (own tokens) and c (shifted grid) per quarter,
        #      with the gating elementwise prep trailing on DVE/ScalarE ----
        # CG = cols of c/u actually consumed by a quarter's gating windows.
        CG = QT + HALO - 1            # 259
        aqs, cqs, uqs, hsss = [], [], [], []
        for q in range(NQ):
            xs = xq_sb[q]
            aq = abpool.tile([128, MH, QT], BF, tag="aq")
            aqs.append(aq)
            cq = abpool.tile([128, MH, CG], BF, tag="cq")
            cqs.append(cq)
            for m in range(MH):
                ps = ps_acc.tile([128, QT], FP32, tag="acc")
                for k in range(KD):
                    nc.tensor.matmul(
                        ps, w1a_sb[:, m, k, :], xs[:, k, HALO:HALO + QT],
                        start=(k == 0), stop=(k == KD - 1),
                    )
                if use_b1:
                    nc.scalar.activation(
                        out=aq[:, m, :], in_=ps, func=AF.Identity,
                        bias=b1_sb[:, m:m + 1], scale=1.0,
                    )
                else:
                    nc.scalar.activation(out=aq[:, m, :], in_=ps, func=AF.Copy)
            for m in range(MH):
                ps = ps_acc.tile([128, CG], FP32, tag="acc")
                for k in range(KD):
                    nc.tensor.matmul(
                        ps, w1c_sb[:, m, k, :], xs[:, k, 0:CG],
                        start=(k == 0), stop=(k == KD - 1),
                    )
                nc.scalar.activation(out=cq[:, m, :], in_=ps, func=AF.Copy)
            # hid = silu(a + c_shift) directly on ScalarE (silu_and_others
            # set also covers tanh/copy/square -> single table load).
            hq = []
            for p in range(W // 2):
                hs = mpool2.tile([128, MH, 2, QT], BF, tag="hs")
                for wi in range(2):
                    w = 2 * p + wi
                    o = HALO - 1 - w
                    nc.vector.tensor_add(hs[:, :, wi, :], aq, cq[:, :, o:o + QT])
                hss = hpool.tile([128, MH, 2, QT], BF, tag="hss")
                nc.scalar.activation(out=hss, in_=hs, func=AF.Silu)
                hq.append(hss)
            hsss.append(hq)
        # ---- stage 1b: u = x @ Wc (+ wv_b @ m2), shifted grid ----
        for q in range(NQ):
            xs = xq_sb[q]
            uq = abpool.tile([128, MD, CG], BF, tag="uq")
            uqs.append(uq)
            for m in range(MD):
                ps = ps_acc.tile([128, CG], FP32, tag="acc")
                for k in range(KD):
                    nc.tensor.matmul(
                        ps, wc_sb[:, m, k, :], xs[:, k, 0:CG],
                        start=(k == 0), stop=(k == KD - 1),
                    )
                if use_bc:
                    nc.scalar.activation(
                        out=uq[:, m, :], in_=ps, func=AF.Identity,
                        bias=bc_sb[:, m:m + 1], scale=1.0,
                    )
                else:
                    nc.vector.tensor_scalar_mul(uq[:, m, :], ps, 1.0)
        # ---- stage 2: per quarter gate -> msg2 -> y -> LN -> store ----
        # Each quarter's LayerNorm finalize is DEFERRED until after the
        # NEXT quarter's msg products: the DVE queue is strictly in-order,
        # and LN ops emitted before the next products would delay msgq,
        # stall the PE transposes, and re-throttle HAM (cold matmuls).
        def emit_ln(g0, group, srow, sqs, ysb):
            nt = len(group)
            ve = nc.vector
            ssum = mpool.tile([128, nt], FP32, tag="ssum", name="ssum")
            ve.tensor_add(ssum, srow[:, :, 0], srow[:, :, 1])
            qsum = mpool.tile([128, nt], FP32, tag="qsum", name="qsum")
            ve.tensor_add(qsum, sqs[:, :, 0], sqs[:, :, 1])
            mean = mpool.tile([128, nt], FP32, tag="mean", name="mean")
            ve.tensor_scalar_mul(mean, ssum, 1.0 / D)
            m2e = mpool.tile([128, nt], FP32, tag="m2e", name="m2e")
            ve.scalar_tensor_tensor(   # mean^2 - eps
                out=m2e, in0=mean, scalar=1.0, in1=mean,
                op0=ALU.mult, op1=ALU.mult,
            )
            ve.tensor_scalar_add(m2e, m2e, -EPS)
            veps = mpool.tile([128, nt], FP32, tag="veps", name="veps")
            ve.scalar_tensor_tensor(   # q/D - (mean^2 - eps)
                out=veps, in0=qsum, scalar=1.0 / D, in1=m2e,
                op0=ALU.mult, op1=ALU.subtract,
            )
            # rstd via bit-trick seed + 2 Newton steps (no sqrt table).
            rbits = mpool.tile([128, nt], I32, tag="rbits", name="rbits")
            ve.tensor_scalar(
                out=rbits, in0=veps.bitcast(I32), scalar1=one_i[:, 0:1],
                scalar2=None, op0=ALU.arith_shift_right,
            )
            ve.tensor_tensor(
                out=rbits, in0=magic_sb.to_broadcast([128, nt]), in1=rbits,
                op=ALU.subtract,
            )
            rstd = rbits.bitcast(FP32)
            for _ in range(2):
                nt1 = mpool.tile([128, nt], FP32, tag="nt1", name="nt1")
                ve.tensor_mul(nt1, rstd, rstd)
                ve.tensor_mul(nt1, nt1, veps)
                ve.tensor_scalar(
                    out=nt1, in0=nt1, scalar1=-0.5, scalar2=1.5,
                    op0=ALU.mult, op1=ALU.add,
                )
                ve.tensor_mul(rstd, rstd, nt1)
            for gi, tt in enumerate(group):
                tok0 = g0 + 128 * tt
                yo = opool.tile([128, D], BF, tag="yout", name="yout")
                ve.tensor_scalar(
                    out=yo, in0=ysb[tt], scalar1=mean[:, gi:gi + 1],
                    scalar2=rstd[:, gi:gi + 1],
                    op0=ALU.subtract, op1=ALU.mult,
                )
                if use_gamma_beta:
                    ve.tensor_mul(yo, yo, gam_sb)
                    ve.tensor_add(yo, yo, bet_sb)
                nc.sync.dma_start(out=y[tok0:tok0 + 128, :], in_=yo)

        pending = []
        for q in range(NQ):
            aq, cq, uq = aqs[q], cqs[q], uqs[q]
            tauq = qpool.tile([128, W, QT], BF, tag="tauq")
            for p in range(W // 2):
                hss = hsss[q][p]
                pl = ps_y.tile([128, 2 * QT], FP32, tag="logit", bufs=1)
                for k in range(MH):
                    nc.tensor.matmul(
                        pl, w2rep_sb[:, k, :], hss[:, k, :, :],
                        start=(k == 0), stop=(k == MH - 1),
                    )
                # tau = sigmoid(logit + b2) = 0.5*tanh(0.5*(logit + b2)) + 0.5
                # (tanh lives in the silu table set; sigmoid does not).
                # b2r is pre-halved on the host; affine runs 4x on DVE.
                traw = mpool2.tile([128, 2, QT], BF, tag="traw")
                nc.scalar.activation(
                    out=traw, in_=pl.rearrange("p (a b) -> p a b", a=2),
                    func=AF.Tanh, bias=b2_sb[:, 0:1], scale=0.5,
                )
                nc.vector.tensor_scalar(
                    out=tauq[:, 2 * p:2 * p + 2, :], in0=traw,
                    scalar1=0.5, scalar2=0.5, op0=ALU.mult, op1=ALU.add,
                )
            # msg2 = sum_w tau_w * shift(u, w+1): tau broadcast over the
            # 8 d-tiles via a step-0 mid dimension. Computed per 128-token
            # tile so the first transpose matmuls unblock after half the
            # DVE chain.
            msgq = qpool.tile([128, MD, QT], BF, tag="msgq")

            def tau_b(w, t0, tauq=tauq):
                s = tauq[:, w, t0:t0 + 128]
                return bass.AP(
                    tensor=s.tensor, offset=s.offset,
                    ap=[s.ap[0], [0, MD], s.ap[1]],
                )

            for tt in range(NT):
                t0 = 128 * tt
                pw = []
                for w in range(W):
                    o = HALO - 1 - w
                    pt = mpool.tile([128, MD, 128], BF, tag="pw")
                    nc.vector.tensor_mul(
                        pt, tau_b(w, t0), uq[:, :, o + t0:o + t0 + 128]
                    )
                    pw.append(pt)
                    if w == 1:
                        m01 = mpool.tile([128, MD, 128], BF, tag="pw")
                        nc.vector.tensor_add(m01, pw[0], pw[1])
                nc.vector.tensor_add(pw[3], pw[2], pw[3])
                nc.vector.tensor_add(msgq[:, :, t0:t0 + 128], m01, pw[3])
            # previous quarter's LayerNorm lands here: after this quarter's
            # products in the DVE queue, before this quarter's y matmuls.
            for job in pending:
                emit_ln(*job)
            pending = []
            # y = x @ m1 + msg2^T (+ merge_b), token-major via PSUM:
            # identity-rhs matmuls transpose-accumulate msg2 into the m1 sum.
            # The last quarter runs per token tile (separate stat tiles) so
            # tt0's LayerNorm + store overlap tt1's matmuls -> shorter tail.
            g0 = q * QT
            tt_groups = [[0, 1]] if q < NQ - 1 else [[0], [1]]
            for group in tt_groups:
                nt = len(group)
                srow = mpool.tile([128, nt, 2], FP32, tag="srow", name="srow")
                sqs = mpool.tile([128, nt, 2], FP32, tag="sqs", name="sqs")
                ysb = {}
                for gi, tt in enumerate(group):
                    yt = opool.tile([128, D], FP32, tag="ysb", name="ysb")
                    ysb[tt] = yt
                    for half in range(2):
                        n0 = half * 512
                        yps = ps_y.tile([128, 512], FP32, tag="y", name="yps")
                        for k in range(KD):
                            nc.tensor.matmul(
                                yps,
                                xq_sb[q][:, k, HALO + 128 * tt:HALO + 128 * tt + 128],
                                m1_sb[:, k, n0:n0 + 512],
                                start=(k == 0), stop=False,
                            )
                        for j in range(4):
                            f = 4 * half + j
                            nc.tensor.matmul(
                                yps[:, 128 * j:128 * j + 128],
                                msgq[:, f, 128 * tt:128 * tt + 128],
                                id_sb,
                                start=False, stop=True,
                            )
                        if use_merge_b:
                            nc.vector.tensor_add(yps, yps, mb_sb[:, n0:n0 + 512])
                        # Evict PSUM while collecting LN stats: Copy gives
                        # sum(y), Square gives sum(y^2); both in silu set.
                        nc.scalar.activation(
                            out=yt[:, n0:n0 + 512], in_=yps, func=AF.Copy,
                            accum_out=srow[:, gi, half:half + 1],
                        )
                        junk = mpool2.tile([128, 512], FP32, tag="junk")
                        nc.scalar.activation(
                            out=junk, in_=yps, func=AF.Square,
                            accum_out=sqs[:, gi, half:half + 1],
                        )
                if q < NQ - 1:
                    pending.append((g0, group, srow, sqs, ysb))
                else:
                    emit_ln(g0, group, srow, sqs, ysb)
    nc.compile()
    return nc


_CACHE: dict = {}


def _get_nc(flags):
    if flags not in _CACHE:
        _CACHE[flags] = build_nc(*flags)
    return _CACHE[flags]


def _pack_km(wmat, mt):
    """[D, mt*128] weight -> [128, mt, KD, 128] m-outer partition-major."""
    # w[k*128+p, m*128+c] -> out[p, m, k, c]
    wr = wmat.reshape(KD, 128, mt, 128)
    return np.ascontiguousarray(wr.transpose(1, 2, 0, 3)).astype(BF16)


def kernel(x, w1, b1, w2, b2, wv_w, wv_b, merge_w, merge_b, gamma, beta):
    x = np.asarray(x, dtype=np.float32)
    w1 = np.asarray(w1, dtype=np.float32)
    b1 = np.asarray(b1, dtype=np.float32)
    w2 = np.asarray(w2, dtype=np.float32)
    b2 = np.asarray(b2, dtype=np.float32)
    wv_w = np.asarray(wv_w, dtype=np.float32)
    wv_b = np.asarray(wv_b, dtype=np.float32)
    merge_w = np.asarray(merge_w, dtype=np.float32)
    merge_b = np.asarray(merge_b, dtype=np.float32)
    gamma = np.asarray(gamma, dtype=np.float32)
    beta = np.asarray(beta, dtype=np.float32)

    m2w = merge_w[D:]                       # [D, D]
    wc = wv_w @ m2w                         # fused value+merge projection
    bc = wv_b @ m2w                         # [D]
    use_gamma_beta = not (np.all(gamma == 1.0) and np.all(beta == 0.0))
    use_merge_b = bool(np.any(merge_b != 0.0))
    use_b1 = bool(np.any(b1 != 0.0))
    use_bc = bool(np.any(bc != 0.0))
    flags = (use_gamma_beta, use_merge_b, use_b1, use_bc)
    nc = _get_nc(flags)

    x2 = x.reshape(B * T, D)
    shared = {
        "w1a": _pack_km(w1[:D], MH),
        "w1c": _pack_km(w1[D:], MH),
        "wc": _pack_km(wc, MD),
        "m1": np.ascontiguousarray(
            merge_w[:D].reshape(KD, 128, D).transpose(1, 0, 2)
        ).astype(BF16),
        "w2rep": np.ascontiguousarray(
            np.broadcast_to(w2.reshape(MH, 128, 1), (MH, 128, 128))
            .transpose(1, 0, 2)
        ).astype(BF16),
        # pre-halved: tau = 0.5*tanh(0.5*logit + 0.5*b2) + 0.5
        "b2r": np.full((128, 1), 0.5 * float(b2[0]), np.float32),
        "ident": np.eye(128, dtype=BF16),
    }
    if use_b1:
        shared["b1r"] = np.ascontiguousarray(b1.reshape(MH, 128).T)
    if use_bc:
        shared["bcr"] = np.ascontiguousarray(bc.reshape(MD, 128).T)
    if use_gamma_beta:
        shared["gam"] = gamma.reshape(1, D)
        shared["bet"] = beta.reshape(1, D)
    if use_merge_b:
        shared["mbt"] = merge_b.reshape(1, D)

    in_maps = []
    for c in range(NCORES):
        t0 = c * NTOK
        xs = np.zeros((NTOK + HALO, D), np.float32)
        xs[HALO:] = x2[t0:t0 + NTOK]
        if t0 % T != 0:  # halo stays inside the same batch element
            xs[:HALO] = x2[t0 - HALO:t0]
        xsT = xs.T.reshape(KD, 128, NTOK + HALO)  # [k, p, grid]
        m = dict(shared)
        for q in range(NQ):
            g0 = q * QT
            m[f"xq{q}"] = np.ascontiguousarray(
                xsT[:, :, g0:g0 + QG].transpose(1, 0, 2)
            ).astype(BF16)
        in_maps.append(m)

    res = run_bass_kernel_spmd(nc, in_maps, core_ids=list(range(NCORES)))
    out = np.concatenate(
        [np.asarray(r["y"]).astype(np.float32) for r in res.results], axis=0
    )
    return out.reshape(B, T, D)
